# revision 1
# baseline (speedup 1.0000x reference)
"""Trainium2 Bass kernel for nn_AttnProcessor (SDXL-style cross-attention with
region-prompt bias coupled through a global score max).

Sharding: data-parallel over batch -- core b handles batch element b (B=8 on 8
cores).  The global max of the attention scores couples the cores, resolved
with an on-device AllReduce(max) of one scalar.

Per-core math (hs [S,D], ehs [L,C], region [S,L]):
  qT[d,s]   = (Wq.T @ hs.T) * SCALE          (fp16 matmuls, fp32 accum)
  kT[d,l]   = Wk.T @ ehs.T ;  v[l,d] = ehs @ Wv
  scT[l,s]  = kT_h.T @ qT_h                  (per head, PSUM fp32)
  gmax      = AllReduce-max over all scT
  sc'       = scT + region.T * (log1p(.1*sigma)*gmax)
  probs     = exp(sc' - C) / sum_l exp(sc' - C)   (C = gmax*(1+log1p) const)
  attnT     = v_h.T @ probs ;  out = attnT.T @ Wo + bo

All matmul operands are fp16 (PSUM accumulation is fp32); elementwise math,
reductions and the bias path stay fp32.  Scores bounce through DRAM in fp16
(chunk-major layout) between the two passes.
"""
import numpy as np

import concourse.bass as bass
import concourse.mybir as mybir
import concourse.tile as tile
from concourse import bacc
from concourse.bass_utils import run_bass_kernel_spmd
from concourse.masks import make_identity

B, S, L, D, C_ENC, H = 8, 4096, 77, 1280, 2048, 20
DH = D // H            # 64
SCALE = DH ** -0.5     # 0.125
N_CORES = 8
CHUNK = 512
NCH = S // CHUNK       # 8
NJ = D // 128          # 10 hd-tiles
NCT_Q = D // 128       # 10 k-tiles for Q
NCT_KV = C_ENC // 128  # 16 k-tiles for K/V
DSLICES = [(0, 512), (512, 512), (1024, 256)]

f32, f16 = mybir.dt.float32, mybir.dt.float16
AX = mybir.AxisListType.X
AF = mybir.ActivationFunctionType
OP = mybir.AluOpType

_CACHE = {}


def build():
    nc = bacc.Bacc("TRN2", target_bir_lowering=False, debug=False,
                   num_devices=N_CORES)
    hs_d = nc.dram_tensor("hidden_states", [S, D], f32, kind="ExternalInput")
    ehs_d = nc.dram_tensor("encoder_hidden_states", [L, C_ENC], f32, kind="ExternalInput")
    reg_d = nc.dram_tensor("region_state", [S, L], f32, kind="ExternalInput")
    wq_d = nc.dram_tensor("Wq", [D, D], f32, kind="ExternalInput")
    wk_d = nc.dram_tensor("Wk", [C_ENC, D], f32, kind="ExternalInput")
    wv_d = nc.dram_tensor("Wv", [C_ENC, D], f32, kind="ExternalInput")
    wo_d = nc.dram_tensor("Wo", [D, D], f32, kind="ExternalInput")
    bo_d = nc.dram_tensor("bo", [D], f32, kind="ExternalInput")
    sig_d = nc.dram_tensor("sigma", [1], f32, kind="ExternalInput")
    out_d = nc.dram_tensor("out", [S, D], f32, kind="ExternalOutput")

    with tile.TileContext(nc) as tc, nc.allow_low_precision(reason="fp16 matmul kernel"):
        with tc.tile_pool(name="consts", bufs=1) as cpool, \
             tc.tile_pool(name="wpool", bufs=1) as wpool, \
             tc.tile_pool(name="big", bufs=1) as bigp, \
             tc.tile_pool(name="work", bufs=1) as wk, \
             tc.tile_pool(name="ps_big", bufs=4, space="PSUM") as psb, \
             tc.tile_pool(name="ps_med", bufs=2, space="PSUM") as psm, \
             tc.tile_pool(name="ps_sm", bufs=2, space="PSUM") as pss, \
             tc.tile_pool(name="dram", bufs=1, space="DRAM") as dr:

            # ---------------- constants ----------------
            id16 = cpool.tile([128, 128], f16)
            make_identity(nc, id16)
            id32 = cpool.tile([128, 128], f32)
            make_identity(nc, id32)
            ones77c = cpool.tile([77, 1], f16)
            nc.vector.memset(ones77c[:], 1.0)
            ones77sq = cpool.tile([77, 77], f16)
            nc.vector.memset(ones77sq[:], 1.0)
            ones128r = cpool.tile([1, 128], f16)
            nc.vector.memset(ones128r[:], 1.0)

            sig = cpool.tile([1, 1], f32)
            nc.sync.dma_start(out=sig[:], in_=sig_d.ap().rearrange("(o a) -> o a", o=1))
            c0 = cpool.tile([1, 1], f32)   # log1p(0.1*sigma)
            nc.scalar.activation(c0[:], sig[:], AF.Ln, bias=1.0, scale=0.1)

            # bo broadcast [128, D]
            bo16 = cpool.tile([1, D], f16)
            nc.gpsimd.dma_start(out=bo16[:], in_=bo_d.ap().rearrange("(o a) -> o a", o=1))

            # ---------------- phase 0: ehsT, kT, v ----------------
            ehs16 = wk.tile([L, C_ENC], f16)
            nc.gpsimd.dma_start(out=ehs16[:], in_=ehs_d[:])
            ehsT = wk.tile([128, NCT_KV * L], f16)   # [c-part, ct*77+l]
            for ct in range(NCT_KV):
                pt = pss.tile([128, 128], f16, tag="sm")
                nc.tensor.transpose(pt[:, 0:L], ehs16[:, ct * 128:(ct + 1) * 128],
                                    id16[0:L, 0:L])
                nc.vector.tensor_copy(ehsT[:, ct * L:(ct + 1) * L], pt[:, 0:L])

            # Wk tiles (streamed through the shared weight pool)
            wk_t = []
            for ct in range(NCT_KV):
                t = wpool.tile([128, D], f16, tag="w", bufs=18, name=f"wk{ct}")
                nc.gpsimd.dma_start(out=t[:], in_=wk_d[ct * 128:(ct + 1) * 128, :])
                wk_t.append(t)
            kT = wk.tile([128, NJ, L], f16)          # [hd-part, j, l]
            for j in range(NJ):
                pk = psm.tile([128, 512], f32, tag="med")
                for ct in range(NCT_KV):
                    nc.tensor.matmul(pk[:, 0:L],
                                     wk_t[ct][:, j * 128:(j + 1) * 128],
                                     ehsT[:, ct * L:(ct + 1) * L],
                                     start=(ct == 0), stop=(ct == NCT_KV - 1))
                nc.scalar.copy(kT[:, j, :], pk[:, 0:L])

            wv_t = []
            for ct in range(NCT_KV):
                t = wpool.tile([128, D], f16, tag="w", bufs=18, name=f"wv{ct}")
                nc.gpsimd.dma_start(out=t[:], in_=wv_d[ct * 128:(ct + 1) * 128, :])
                wv_t.append(t)
            v_sb = wk.tile([L, D], f16)              # [l, hd]
            for d0, dn in DSLICES:
                pv0 = psm.tile([128, 512], f32, tag="med")
                for ct in range(NCT_KV):
                    nc.tensor.matmul(pv0[0:L, 0:dn],
                                     ehsT[:, ct * L:(ct + 1) * L],
                                     wv_t[ct][:, d0:d0 + dn],
                                     start=(ct == 0), stop=(ct == NCT_KV - 1))
                nc.scalar.copy(v_sb[:, d0:d0 + dn], pv0[0:L, 0:dn])

            # ---------------- regionT (fp32, exact) ----------------
            regT = bigp.tile([L, S], f32)            # 16 KB/partition
            for ci in range(NCH):
                rin = wk.tile([128, 4, L], f32, tag="rin", bufs=1)
                nc.sync.dma_start(
                    out=rin[:],
                    in_=reg_d[ci * CHUNK:(ci + 1) * CHUNK, :]
                        .rearrange("(t p) l -> p t l", p=128))
                for t in range(4):
                    ptr = pss.tile([128, 128], f32, tag="sm")
                    nc.tensor.transpose(ptr[0:L, :], rin[:, t, :], id32[:])
                    nc.vector.tensor_copy(
                        regT[:, ci * CHUNK + t * 128: ci * CHUNK + (t + 1) * 128],
                        ptr[0:L, :])

            # ---------------- Wq tiles ----------------
            wq_t = []
            for ct in range(NCT_Q):
                t = wpool.tile([128, D], f16, tag="w", bufs=18, name=f"wq{ct}")
                nc.gpsimd.dma_start(out=t[:], in_=wq_d[ct * 128:(ct + 1) * 128, :])
                wq_t.append(t)

            # scores DRAM scratch, chunk-major: [l, chunk, head, s-in-chunk]
            sc_dram = dr.tile([L, NCH, H, CHUNK], f16)

            Mx = cpool.tile([128, 1], f32)
            nc.vector.memset(Mx[:], -3.0e38)

            # ---------------- phase 1: qT, scores, local max ----------------
            for ci in range(NCH):
                hs16 = wk.tile([128, 4, D], f16, tag="hs", bufs=2)
                nc.gpsimd.dma_start(
                    out=hs16[:],
                    in_=hs_d[ci * CHUNK:(ci + 1) * CHUNK, :]
                        .rearrange("(t p) c -> p t c", p=128))
                hsT = wk.tile([128, NCT_Q, CHUNK], f16, tag="hsT", bufs=1)
                for t in range(4):
                    for ct in range(NCT_Q):
                        pt = pss.tile([128, 128], f16, tag="sm")
                        nc.tensor.transpose(pt[:], hs16[:, t, ct * 128:(ct + 1) * 128],
                                            id16[:])
                        nc.vector.tensor_copy(hsT[:, ct, t * 128:(t + 1) * 128], pt[:])
                qT = wk.tile([128, NJ, CHUNK], f16, tag="qT", bufs=2)
                for j in range(NJ):
                    pq = psb.tile([128, 512], f32, tag="big")
                    for ct in range(NCT_Q):
                        nc.tensor.matmul(pq[:], wq_t[ct][:, j * 128:(j + 1) * 128],
                                         hsT[:, ct, :],
                                         start=(ct == 0), stop=(ct == NCT_Q - 1))
                    nc.scalar.activation(qT[:, j, :], pq[:], AF.Copy,
                                         bias=0.0, scale=float(SCALE))
                stg = wk.tile([L, H, CHUNK], f16, tag="stg", bufs=2)
                for h in range(H):
                    j, r = divmod(h, 2)
                    psc = psm.tile([128, 512], f32, tag="med")
                    nc.tensor.matmul(psc[0:L, :],
                                     kT[r * 64:(r + 1) * 64, j, :],
                                     qT[r * 64:(r + 1) * 64, j, :],
                                     start=True, stop=True)
                    nc.scalar.copy(stg[:, h, :], psc[0:L, :])
                mloc = wk.tile([L, 1], f32, tag="mloc", bufs=4)
                nc.vector.reduce_max(mloc[:], stg[:].rearrange("p a b -> p (a b)"),
                                     axis=AX)
                nc.vector.tensor_tensor(out=Mx[0:L, :], in0=Mx[0:L, :],
                                        in1=mloc[:], op=OP.max)
                nc.sync.dma_start(out=sc_dram[:, ci, :, :], in_=stg[:])

            # ---------------- global max + constants ----------------
            pmx = pss.tile([128, 128], f32, tag="sm")
            nc.tensor.transpose(pmx[0:1, :], Mx[:], id32[:])
            gmx = cpool.tile([1, 1], f32)
            nc.vector.reduce_max(gmx[:], pmx[0:1, :], axis=AX)

            cin = dr.tile([1, 1], f32)
            cout = dr.tile([1, 1], f32, addr_space="Shared")
            nc.sync.dma_start(out=cin[:], in_=gmx[:])
            nc.gpsimd.collective_compute(
                "AllReduce", OP.max,
                replica_groups=[list(range(N_CORES))],
                ins=[cin.opt()], outs=[cout.opt()])
            gmaxg = cpool.tile([1, 1], f32)
            nc.sync.dma_start(out=gmaxg[:], in_=cout[:])

            cb = cpool.tile([1, 1], f32)     # log1p(0.1 sigma) * gmax
            nc.vector.tensor_tensor(out=cb[:], in0=gmaxg[:], in1=c0[:], op=OP.mult)
            cc = cpool.tile([1, 1], f32)     # C = gmax + cb  (softmax shift)
            nc.vector.tensor_tensor(out=cc[:], in0=gmaxg[:], in1=cb[:], op=OP.add)
            negC = cpool.tile([1, 1], f32)
            nc.vector.tensor_scalar_mul(negC[:], cc[:], -1.0)

            cb_d = dr.tile([1, 1], f32)
            nc.sync.dma_start(out=cb_d[:], in_=cb[:])
            cb_bc = cpool.tile([L, 1], f32)
            nc.sync.dma_start(out=cb_bc[:], in_=cb_d[:].broadcast_to((L, 1)))
            negC_d = dr.tile([1, 1], f32)
            nc.sync.dma_start(out=negC_d[:], in_=negC[:])
            negC_bc = cpool.tile([L, 1], f32)
            nc.sync.dma_start(out=negC_bc[:], in_=negC_d[:].broadcast_to((L, 1)))

            # ---------------- Wo tiles (reuse weight pool slots) ----------------
            wo_t = []
            for ct in range(NJ):
                t = wpool.tile([128, D], f16, tag="w", bufs=18, name=f"wo{ct}")
                nc.gpsimd.dma_start(out=t[:], in_=wo_d[ct * 128:(ct + 1) * 128, :])
                wo_t.append(t)

            # ---------------- phase 2: softmax, PV, output ----------------
            for ci in range(NCH):
                scs = wk.tile([L, H, CHUNK], f16, tag="stg", bufs=2)
                nc.sync.dma_start(out=scs[:], in_=sc_dram[:, ci, :, :])
                rgb = wk.tile([L, CHUNK], f32, tag="rgb", bufs=2)
                nc.vector.tensor_scalar(out=rgb[:], in0=regT[:, ci * CHUNK:(ci + 1) * CHUNK],
                                        scalar1=cb_bc[:], scalar2=None, op0=OP.mult)
                # exp(bias - C), once per chunk; probs factorizes as
                # exp(s) * exp(bias - C) * (1/den)
                expb = wk.tile([L, CHUNK], f16, tag="expb", bufs=2)
                nc.scalar.activation(expb[:], rgb[:], AF.Exp,
                                     bias=negC_bc[:], scale=1.0)
                attnT = wk.tile([128, NJ, CHUNK], f16, tag="attnT", bufs=1)
                for h in range(H):
                    j, r = divmod(h, 2)
                    es = wk.tile([L, CHUNK], f16, tag="es", bufs=4)
                    nc.scalar.activation(es[:], scs[:, h, :], AF.Exp)
                    ex = wk.tile([L, CHUNK], f16, tag="exf", bufs=4)
                    nc.vector.tensor_tensor(out=ex[:], in0=es[:], in1=expb[:],
                                            op=OP.mult)
                    pdenb = psm.tile([128, 512], f32, tag="med")
                    nc.tensor.matmul(pdenb[0:L, :], ones77sq[:], ex[:],
                                     start=True, stop=True)
                    rd = wk.tile([L, CHUNK], f16, tag="rd", bufs=4)
                    nc.vector.reciprocal(rd[:], pdenb[0:L, :])
                    probs = wk.tile([L, CHUNK], f16, tag="probs", bufs=3)
                    nc.vector.tensor_tensor(out=probs[:], in0=ex[:],
                                            in1=rd[:], op=OP.mult)
                    ppv = pss.tile([128, 512], f32, tag="sm")
                    nc.tensor.matmul(ppv[r * 64:(r + 1) * 64, :],
                                     v_sb[:, h * 64:(h + 1) * 64], probs[:],
                                     start=True, stop=True)
                    nc.scalar.copy(attnT[r * 64:(r + 1) * 64, j, :],
                                   ppv[r * 64:(r + 1) * 64, :])
                out_sb = wk.tile([128, D], f32, tag="osb", bufs=2)
                for st in range(4):
                    for d0, dn in DSLICES:
                        pf = psb.tile([128, 512], f32, tag="big")
                        for j in range(NJ):
                            nc.tensor.matmul(pf[:, 0:dn],
                                             attnT[:, j, st * 128:(st + 1) * 128],
                                             wo_t[j][:, d0:d0 + dn],
                                             start=(j == 0), stop=False)
                        nc.tensor.matmul(pf[:, 0:dn], ones128r[:],
                                         bo16[:, d0:d0 + dn],
                                         start=False, stop=True)
                        nc.vector.tensor_copy(out_sb[:, d0:d0 + dn], pf[:, 0:dn])
                    nc.sync.dma_start(
                        out=out_d[ci * CHUNK + st * 128: ci * CHUNK + (st + 1) * 128, :],
                        in_=out_sb[:])
                    if st < 3:
                        out_sb = wk.tile([128, D], f32, tag="osb", bufs=2)
    nc.compile()
    return nc


def kernel(hidden_states, encoder_hidden_states, region_state,
           Wq, Wk, Wv, Wo, bo, sigma):
    if "nc" not in _CACHE:
        _CACHE["nc"] = build()
    nc = _CACHE["nc"]
    in_maps = []
    for b in range(N_CORES):
        in_maps.append({
            "hidden_states": np.ascontiguousarray(hidden_states[b], dtype=np.float32),
            "encoder_hidden_states": np.ascontiguousarray(encoder_hidden_states[b], dtype=np.float32),
            "region_state": np.ascontiguousarray(region_state[b], dtype=np.float32),
            "Wq": np.asarray(Wq, dtype=np.float32),
            "Wk": np.asarray(Wk, dtype=np.float32),
            "Wv": np.asarray(Wv, dtype=np.float32),
            "Wo": np.asarray(Wo, dtype=np.float32),
            "bo": np.asarray(bo, dtype=np.float32),
            "sigma": np.asarray(sigma, dtype=np.float32),
        })
    r = run_bass_kernel_spmd(nc, in_maps, list(range(N_CORES)))
    return np.stack([r.results[c]["out"] for c in range(N_CORES)], axis=0)



# revision 6
# speedup vs baseline: 144.5225x; 144.5225x over previous
"""Trainium2 Bass kernel for nn_AttnProcessor (SDXL-style cross-attention with
region-prompt bias coupled through a global score max) — optimized v2.

Sharding: data-parallel over batch — core b handles batch element b (B=8 on 8
cores).  The global max of the attention scores couples the cores, resolved
with an on-device AllReduce(max) of one scalar.

HW-profiling-driven changes vs the baseline kernel (wall-slope bisect showed
phase 2 at ~1.46 ms of the 1.62 ms total; phase 1 at ~150 us):
  - Phase 2: ACT runs ONLY Exp (no per-head PSUM->SBUF copies on ACT, so no
    activation-table thrashing).  Denominators for a head pair are built by
    two ones[77,64]-stationary matmuls into one [128,512] PSUM tile (rows
    0-63 = den of head 2j, 64-127 = den of head 2j+1), reciprocal'd once per
    pair, and applied by DVE as the PSUM->SBUF evacuation of the unnormalized
    PV result:  attnT_j = (v.T @ ex) * rdj.  This removes the per-head
    [77,512] reciprocal+mult chain and the ACT copy.
  - Output projection: bias via the ones-row matmul into PSUM, then DMA
    straight from PSUM to DRAM (no SBUF staging, no DVE evacuation).
  - Weights DMA'd in 4-tile packs (14 casting DMAs instead of 52; SWDGE
    issue overhead is ~1 us each on the single gpsimd queue).
  - Phase-1 local max split per j-pair so no monolithic reduce blocks DVE.
  - Phase-1 scores interleaved into the qT j-loop; hsT double-buffered.
  - region held/transposed in fp16.

All matmul operands are fp16 (PSUM accumulation is fp32).  Scores bounce
through DRAM in fp16 between the two passes.

build(loop=K) wraps the body in a hardware For_i loop for device-time
measurement by wall-clock slope (the axon dispatch floor is ~70-130 ms, so a
single execution's device time is unobservable from wall clock).
build(collective=False) replaces the AllReduce with a local copy (the
collective cannot be re-executed inside a hardware loop; timing variant only).
"""
import contextlib
import numpy as np

import concourse.bass as bass
import concourse.mybir as mybir
import concourse.tile as tile
from concourse import bacc
from concourse.bass_utils import run_bass_kernel_spmd
from concourse.masks import make_identity

B, S, L, D, C_ENC, H = 8, 4096, 77, 1280, 2048, 20
DH = D // H            # 64
SCALE = DH ** -0.5     # 0.125
N_CORES = 8
CHUNK = 512
NCH = S // CHUNK       # 8
NJ = D // 128          # 10 hd-tiles
NCT_Q = D // 128       # 10 k-tiles for Q
NCT_KV = C_ENC // 128  # 16 k-tiles for K/V
DSLICES = [(0, 512), (512, 512), (1024, 256)]
WPACK = 4              # 128-col tile groups per weight DMA

f32, f16 = mybir.dt.float32, mybir.dt.float16
AX = mybir.AxisListType.X
AF = mybir.ActivationFunctionType
OP = mybir.AluOpType

_CACHE = {}


def build(loop: int = 1, collective: bool = True):
    nc = bacc.Bacc("TRN2", target_bir_lowering=False, debug=False,
                   num_devices=N_CORES)
    hs_d = nc.dram_tensor("hidden_states", [S, D], f32, kind="ExternalInput")
    ehs_d = nc.dram_tensor("encoder_hidden_states", [L, C_ENC], f32, kind="ExternalInput")
    reg_d = nc.dram_tensor("region_state", [S, L], f32, kind="ExternalInput")
    wq_d = nc.dram_tensor("Wq", [D, D], f32, kind="ExternalInput")
    wk_d = nc.dram_tensor("Wk", [C_ENC, D], f32, kind="ExternalInput")
    wv_d = nc.dram_tensor("Wv", [C_ENC, D], f32, kind="ExternalInput")
    wo_d = nc.dram_tensor("Wo", [D, D], f32, kind="ExternalInput")
    bo_d = nc.dram_tensor("bo", [D], f32, kind="ExternalInput")
    sig_d = nc.dram_tensor("sigma", [1], f32, kind="ExternalInput")
    out_d = nc.dram_tensor("out", [S, D], f32, kind="ExternalOutput")

    with tile.TileContext(nc) as tc, nc.allow_low_precision(reason="fp16 matmul kernel"):
        with tc.tile_pool(name="consts", bufs=1) as cpool, \
             tc.tile_pool(name="wpool", bufs=1) as wpool, \
             tc.tile_pool(name="big", bufs=1) as bigp, \
             tc.tile_pool(name="work", bufs=1) as wk, \
             tc.tile_pool(name="ps_acc", bufs=1, space="PSUM") as psA, \
             tc.tile_pool(name="ps_med", bufs=1, space="PSUM") as psB, \
             tc.tile_pool(name="ps_sm", bufs=1, space="PSUM") as psC, \
             tc.tile_pool(name="dram", bufs=1, space="DRAM") as dr, \
             (tc.For_i(0, loop, 1) if loop > 1 else contextlib.nullcontext()):

            # ---------------- constants ----------------
            id16 = cpool.tile([128, 128], f16)
            make_identity(nc, id16)
            id32 = cpool.tile([128, 128], f32)
            make_identity(nc, id32)
            ones77h = cpool.tile([77, 64], f16)
            nc.vector.memset(ones77h[:], 1.0)
            ones128r = cpool.tile([1, 128], f16)
            nc.vector.memset(ones128r[:], 1.0)

            sig = cpool.tile([1, 1], f32)
            nc.sync.dma_start(out=sig[:], in_=sig_d.ap().rearrange("(o a) -> o a", o=1))
            c0 = cpool.tile([1, 1], f32)   # log1p(0.1*sigma)
            nc.scalar.activation(c0[:], sig[:], AF.Ln, bias=1.0, scale=0.1)

            # ---------------- weight loads (packed casting DMAs) ----------------
            # gpsimd queue order matters: Wq + first hs chunks first (phase-1
            # critical path), then Wk (kT), region, Wv, Wo, bo.
            ehs16 = wk.tile([L, C_ENC], f16, tag="stg", bufs=2)
            nc.gpsimd.dma_start(out=ehs16[:], in_=ehs_d[:])

            def wload(dram, n128, name):
                tiles = []
                for p0 in range(0, n128, WPACK):
                    pn = min(WPACK, n128 - p0)
                    t = wpool.tile([128, WPACK, D], f16, tag="w", bufs=7,
                                   name=f"{name}{p0}")
                    nc.gpsimd.dma_start(
                        out=t[:, 0:pn, :],
                        in_=dram[p0 * 128:(p0 + pn) * 128, :]
                            .rearrange("(a p) c -> p a c", p=128))
                    tiles.append(t)
                return lambda ct: tiles[ct // WPACK][:, ct % WPACK, :]

            wq_at = wload(wq_d, NCT_Q, "wq")

            hs16s = {}
            for ci in range(2):       # pre-issue first two hs chunks
                hs16s[ci] = wk.tile([128, 4, D], f16, tag="hs", bufs=2,
                                    name=f"hs16_{ci}")
                nc.gpsimd.dma_start(
                    out=hs16s[ci][:],
                    in_=hs_d[ci * CHUNK:(ci + 1) * CHUNK, :]
                        .rearrange("(t p) c -> p t c", p=128))

            wk_at = wload(wk_d, NCT_KV, "wk")

            for ci in range(NCH):     # region (used only in phase 2)
                rin = wk.tile([128, 4, L], f16, tag="rin", bufs=2)
                nc.gpsimd.dma_start(
                    out=rin[:],
                    in_=reg_d[ci * CHUNK:(ci + 1) * CHUNK, :]
                        .rearrange("(t p) l -> p t l", p=128))
                hs16s[f"r{ci}"] = rin

            wv_at = wload(wv_d, NCT_KV, "wv")
            wo_at = wload(wo_d, NJ, "wo")

            bo16 = cpool.tile([1, D], f16)
            nc.gpsimd.dma_start(out=bo16[:], in_=bo_d.ap().rearrange("(o a) -> o a", o=1))

            # ---------------- phase 0: ehsT, kT, regT, v ----------------
            ehsT = wk.tile([128, NCT_KV * L], f16)   # [c-part, ct*77+l]
            for ct in range(NCT_KV):
                pt = psC.tile([128, 128], f16, tag="a", bufs=4)
                nc.tensor.transpose(pt[:, 0:L], ehs16[:, ct * 128:(ct + 1) * 128],
                                    id16[0:L, 0:L])
                nc.vector.tensor_copy(ehsT[:, ct * L:(ct + 1) * L], pt[:, 0:L])

            kT = wk.tile([128, NJ, L], f16)          # [hd-part, j, l]
            for j in range(NJ):
                pk = psA.tile([128, 2, 512], f32, tag="b", bufs=2)
                for ct in range(NCT_KV):
                    nc.tensor.matmul(pk[:, 0, 0:L],
                                     wk_at(ct)[:, j * 128:(j + 1) * 128],
                                     ehsT[:, ct * L:(ct + 1) * L],
                                     start=(ct == 0), stop=(ct == NCT_KV - 1))
                nc.scalar.copy(kT[:, j, :], pk[:, 0, 0:L])

            regT = bigp.tile([L, S], f16)            # 8 KB/partition
            for ci in range(NCH):
                rin = hs16s[f"r{ci}"]
                for t in range(4):
                    ptr = psC.tile([128, 128], f16, tag="a", bufs=4)
                    nc.tensor.transpose(ptr[0:L, :], rin[:, t, :], id16[:])
                    nc.vector.tensor_copy(
                        regT[:, ci * CHUNK + t * 128: ci * CHUNK + (t + 1) * 128],
                        ptr[0:L, :])

            v_sb = wk.tile([L, D], f16)              # [l, hd]
            for d0, dn in DSLICES:
                pv0 = psA.tile([128, 2, 512], f32, tag="b", bufs=2)
                for ct in range(NCT_KV):
                    nc.tensor.matmul(pv0[0:L, 0, 0:dn],
                                     ehsT[:, ct * L:(ct + 1) * L],
                                     wv_at(ct)[:, d0:d0 + dn],
                                     start=(ct == 0), stop=(ct == NCT_KV - 1))
                nc.scalar.copy(v_sb[:, d0:d0 + dn], pv0[0:L, 0, 0:dn])

            # scores DRAM scratch, chunk-major: [l, chunk, head, s-in-chunk]
            sc_dram = dr.tile([L, NCH, H, CHUNK], f16)

            Mx = cpool.tile([128, 1], f32)
            nc.vector.memset(Mx[:], -3.0e38)

            # ---------------- phase 1: qT, scores, local max ----------------
            for ci in range(NCH):
                if ci in hs16s:
                    hs16 = hs16s[ci]
                else:
                    hs16 = wk.tile([128, 4, D], f16, tag="hs", bufs=2)
                    nc.gpsimd.dma_start(
                        out=hs16[:],
                        in_=hs_d[ci * CHUNK:(ci + 1) * CHUNK, :]
                            .rearrange("(t p) c -> p t c", p=128))
                hsT = wk.tile([128, NCT_Q, CHUNK], f16, tag="hsT", bufs=2)
                for t in range(4):
                    for ct in range(NCT_Q):
                        pt = psC.tile([128, 128], f16, tag="a", bufs=4)
                        nc.tensor.transpose(pt[:], hs16[:, t, ct * 128:(ct + 1) * 128],
                                            id16[:])
                        nc.vector.tensor_copy(hsT[:, ct, t * 128:(t + 1) * 128], pt[:])
                stg = wk.tile([L, H, CHUNK], f16, tag="stg", bufs=2)
                for j in range(NJ):
                    pq = psA.tile([128, 2, 512], f32, tag="b", bufs=2)
                    for ct in range(NCT_Q):
                        nc.tensor.matmul(pq[:, 0, :], wq_at(ct)[:, j * 128:(j + 1) * 128],
                                         hsT[:, ct, :],
                                         start=(ct == 0), stop=(ct == NCT_Q - 1))
                    qTj = wk.tile([128, CHUNK], f16, tag="qT", bufs=2)
                    nc.scalar.activation(qTj[:], pq[:, 0, :], AF.Copy,
                                         bias=0.0, scale=float(SCALE))
                    for r in range(2):
                        h = 2 * j + r
                        psc = psC.tile([128, 512], f32, tag="a", bufs=4)
                        nc.tensor.matmul(psc[0:L, :],
                                         kT[r * 64:(r + 1) * 64, j, :],
                                         qTj[r * 64:(r + 1) * 64, :],
                                         start=True, stop=True)
                        nc.scalar.copy(stg[:, h, :], psc[0:L, :])
                    mloc = wk.tile([L, 1], f32, tag="mloc", bufs=3)
                    nc.vector.reduce_max(
                        mloc[:], stg[:, 2 * j:2 * j + 2, :].rearrange("p a b -> p (a b)"),
                        axis=AX)
                    nc.vector.tensor_tensor(out=Mx[0:L, :], in0=Mx[0:L, :],
                                            in1=mloc[:], op=OP.max)
                nc.sync.dma_start(out=sc_dram[:, ci, :, :], in_=stg[:])

            # ---------------- global max + constants ----------------
            pmx = psC.tile([128, 128], f32, tag="a", bufs=4)
            nc.tensor.transpose(pmx[0:1, :], Mx[:], id32[:])
            gmx = cpool.tile([1, 1], f32)
            nc.vector.reduce_max(gmx[:], pmx[0:1, :], axis=AX)

            cin = dr.tile([1, 1], f32)
            cout = dr.tile([1, 1], f32, addr_space="Shared")
            nc.sync.dma_start(out=cin[:], in_=gmx[:])
            if collective:
                nc.gpsimd.collective_compute(
                    "AllReduce", OP.max,
                    replica_groups=[list(range(N_CORES))],
                    ins=[cin.opt()], outs=[cout.opt()])
            else:
                nc.sync.dma_start(out=cout[:], in_=cin[:])
            gmaxg = cpool.tile([1, 1], f32)
            nc.sync.dma_start(out=gmaxg[:], in_=cout[:])

            cb = cpool.tile([1, 1], f32)     # log1p(0.1 sigma) * gmax
            nc.vector.tensor_tensor(out=cb[:], in0=gmaxg[:], in1=c0[:], op=OP.mult)
            cc = cpool.tile([1, 1], f32)     # C = gmax + cb  (softmax shift)
            nc.vector.tensor_tensor(out=cc[:], in0=gmaxg[:], in1=cb[:], op=OP.add)
            negC = cpool.tile([1, 1], f32)
            nc.vector.tensor_scalar_mul(negC[:], cc[:], -1.0)

            cb_d = dr.tile([1, 1], f32)
            nc.sync.dma_start(out=cb_d[:], in_=cb[:])
            cb_bc = cpool.tile([L, 1], f32)
            nc.sync.dma_start(out=cb_bc[:], in_=cb_d[:].broadcast_to((L, 1)))
            negC_d = dr.tile([1, 1], f32)
            nc.sync.dma_start(out=negC_d[:], in_=negC[:])
            negC_bc = cpool.tile([L, 1], f32)
            nc.sync.dma_start(out=negC_bc[:], in_=negC_d[:].broadcast_to((L, 1)))

            # rgbC = region^T * cb, fp16, in place over regT
            nc.vector.tensor_scalar(out=regT[:], in0=regT[:],
                                    scalar1=cb_bc[:], scalar2=None, op0=OP.mult)

            # ---------------- phase 2: softmax, PV, output ----------------
            # probs factorization: exp(qk + reg*cb - C) = exp(qk) * expb where
            # expb = exp(reg*cb - C) is head-independent (one ACT op per chunk).
            # ACT emits ONLY Exp in this phase (no function-set thrashing).
            for ci in range(NCH):
                scs = wk.tile([L, H, CHUNK], f16, tag="stg", bufs=2)
                nc.sync.dma_start(out=scs[:], in_=sc_dram[:, ci, :, :])
                expb = wk.tile([L, CHUNK], f16, tag="expb", bufs=2)
                nc.scalar.activation(expb[:], regT[:, ci * CHUNK:(ci + 1) * CHUNK],
                                     AF.Exp, bias=negC_bc[:], scale=1.0)
                attnT = wk.tile([128, NJ, CHUNK], f16, tag="attnT", bufs=1)
                expb_bc = expb[:].rearrange("p (o f) -> p o f", o=1) \
                                 .broadcast_to((L, 4, CHUNK))
                for g in range(H // 4):
                    es4 = wk.tile([L, 4, CHUNK], f16, tag="es", bufs=1)
                    nc.scalar.activation(es4[:], scs[:, 4 * g:4 * g + 4, :], AF.Exp)
                    ex4 = wk.tile([L, 4, CHUNK], f16, tag="ex", bufs=2)
                    nc.vector.tensor_tensor(out=ex4[:], in0=es4[:], in1=expb_bc,
                                            op=OP.mult)
                    pden4 = psA.tile([128, 2, 512], f32, tag="b", bufs=2)
                    for r in range(4):
                        nc.tensor.matmul(pden4[(r % 2) * 64:(r % 2 + 1) * 64, r // 2, :],
                                         ones77h[:], ex4[:, r, :],
                                         start=True, stop=True)
                    appv4 = psA.tile([128, 2, 512], f32, tag="b", bufs=2)
                    for r in range(4):
                        h = 4 * g + r
                        nc.tensor.matmul(appv4[(r % 2) * 64:(r % 2 + 1) * 64, r // 2, :],
                                         v_sb[:, h * 64:(h + 1) * 64], ex4[:, r, :],
                                         start=True, stop=True)
                    rd4 = wk.tile([128, 2, CHUNK], f16, tag="rdj", bufs=1)
                    nc.vector.reciprocal(rd4[:], pden4[:])
                    nc.vector.tensor_tensor(out=attnT[:, 2 * g:2 * g + 2, :],
                                            in0=appv4[:], in1=rd4[:], op=OP.mult)
                for st in range(4):
                    out_sb = wk.tile([128, D], f32, tag="osb", bufs=2,
                                     name=f"osb{ci}_{st}")
                    for d0, dn in DSLICES:
                        pf = psC.tile([128, 512], f32, tag="a", bufs=4)
                        for j in range(NJ):
                            nc.tensor.matmul(pf[:, 0:dn],
                                             attnT[:, j, st * 128:(st + 1) * 128],
                                             wo_at(j)[:, d0:d0 + dn],
                                             start=(j == 0), stop=False)
                        nc.tensor.matmul(pf[:, 0:dn], ones128r[:],
                                         bo16[:, d0:d0 + dn],
                                         start=False, stop=True)
                        nc.vector.tensor_copy(out_sb[:, d0:d0 + dn], pf[:, 0:dn])
                    nc.sync.dma_start(
                        out=out_d[ci * CHUNK + st * 128: ci * CHUNK + (st + 1) * 128, :],
                        in_=out_sb[:])
    nc.compile()
    return nc


def kernel(hidden_states, encoder_hidden_states, region_state,
           Wq, Wk, Wv, Wo, bo, sigma):
    if "nc" not in _CACHE:
        _CACHE["nc"] = build()
    nc = _CACHE["nc"]
    in_maps = []
    for b in range(N_CORES):
        in_maps.append({
            "hidden_states": np.ascontiguousarray(hidden_states[b], dtype=np.float32),
            "encoder_hidden_states": np.ascontiguousarray(encoder_hidden_states[b], dtype=np.float32),
            "region_state": np.ascontiguousarray(region_state[b], dtype=np.float32),
            "Wq": np.asarray(Wq, dtype=np.float32),
            "Wk": np.asarray(Wk, dtype=np.float32),
            "Wv": np.asarray(Wv, dtype=np.float32),
            "Wo": np.asarray(Wo, dtype=np.float32),
            "bo": np.asarray(bo, dtype=np.float32),
            "sigma": np.asarray(sigma, dtype=np.float32),
        })
    r = run_bass_kernel_spmd(nc, in_maps, list(range(N_CORES)))
    return np.stack([r.results[c]["out"] for c in range(N_CORES)], axis=0)
